# revision 7
# baseline (speedup 1.0000x reference)
"""CAM (channel attention) module kernel for Trainium2 (Bass/Tile) — v3.

Reference computation (per batch b):
    energy  = x_b @ x_b.T                      # [C, C], contraction over N
    att     = softmax(rowmax(energy) - energy) # row-wise over last axis
    out     = att @ x_b                        # [C, N]
    y_b     = gamma * out + x_b

Sharding: data-parallel over B across 8 NeuronCores (B=32 -> 4 per core),
gamma replicated, full CxC attention per core.

Identity used: softmax(rowmax(E) - E)[i,j] = exp(mn[i] - E[i,j]) / Z[i]
with mn[i] = min_j E[i,j], Z[i] = sum_j exp(mn[i] - E[i,j]).

Evidence-driven changes (NTFF profiles of v1/v2):
  * all PE transposes are REGULAR matmuls against an fp8 identity
    (moving operand = identity): ~56ns issue vs ~330-480ns transpose-mode,
    and they keep the PE clock-gate (HAM) warm.
  * fp8e4 operands (TRN e4m3, +-240 range; |x|<~6); mm1+mm2 DoubleRow.
  * batch loop is software-pipelined: batch b+1's input DMA + fp8 cast are
    emitted BEFORE batch b's mm2/output-DMA block.  (v2 queued them after:
    the Sync FIFO's out-DMAs head-of-line-blocked the next input, giving a
    ~15us PE hole + HAM re-throttle at every batch boundary.)
  * x->fp8 cast chunked by column: chunks 0-3 on GpSimd (idle during mm2),
    4,6 on ScalarE, 5,7 on DVE.
  * gamma folded into the t^T evacuation (ACT scale), output evac is one
    scalar_tensor_tensor (mul 1/Z, add residual), outputs DMA'd in 1024-col
    pairs.

Layouts per batch (P=128 partitions):
    X   [P, CO, N]  f32  c-natural  (c = co*P + p)        -- DMA from DRAM
    X8  [P, CO, N]  fp8  cast of X  (mm2 moving operand)
    xt8 [P, 2, C]   fp8  per k-pair, n on partitions      -- PE transpose
    E   [P, CO, C]  PSUM f32, i on partitions             -- mm1 (DoubleRow)
    tS  [P, CO, C]  fp8  exp(mn - E); Z row-sum fused     -- ScalarE
    tT8 [P, CO, C]  fp8  gamma * t^T, j on partitions     -- PE + ACT scale
    out [P, 1024]   f32  (tT8.T @ X8) * (1/Z[i]) + X      -- mm2 (DoubleRow)
"""

import contextlib

import numpy as np

P = 128

_CACHE = {}


DEFAULT_OPTS = dict(
    dr1=True,        # DoubleRow for mm1 (energy)
    dr2=True,        # DoubleRow for mm2 (att @ x)
    xt_bufs=6,       # xt8 k-pair SBUF tiles
    psx_bufs=2,      # PSUM banks for x-transpose staging
    acc_bufs=2,      # PSUM banks shared by t-transpose + matmul-2
    o_bufs=4,        # output staging tiles [P, 1024]
    pipe1=2,         # mm1 trails transposes by this many k-pairs
    cast_engines=("gpsimd", "gpsimd", "gpsimd", "gpsimd",
                  "scalar", "vector", "scalar", "vector"),
)


def _build(Bs, C, N, reps=1, **opts):
    import concourse.bass as bass  # noqa: F401
    import concourse.tile as tile
    import concourse.mybir as mybir
    from concourse import bacc
    from concourse.masks import make_identity

    o = dict(DEFAULT_OPTS)
    o.update(opts)

    F32 = mybir.dt.float32
    BF16 = mybir.dt.bfloat16
    FP8 = mybir.dt.float8e4
    AF = mybir.ActivationFunctionType
    ALU = mybir.AluOpType
    AX = mybir.AxisListType
    DR = mybir.MatmulPerfMode.DoubleRow

    assert C == 4 * P and N % 1024 == 0
    CO = C // P          # i/j chunks of 128
    KC = N // P          # n chunks of 128 (contraction for energy)
    KP = KC // 2         # n chunk-pairs (DoubleRow contraction of 256)
    NF = N // 512        # n chunks of 512

    nc = bacc.Bacc(None, target_bir_lowering=False, debug=False)
    x_in = nc.dram_tensor("x", [Bs, C, N], F32, kind="ExternalInput")
    g_in = nc.dram_tensor("gamma", [1], F32, kind="ExternalInput")
    y_out = nc.dram_tensor("y", [Bs, C, N], F32, kind="ExternalOutput")

    with tile.TileContext(nc) as tc:
        with (
            tc.tile_pool(name="consts", bufs=1) as consts,
            tc.tile_pool(name="xpool", bufs=2) as xpool,
            tc.tile_pool(name="x8pool", bufs=2) as x8pool,
            tc.tile_pool(name="xtp", bufs=o["xt_bufs"]) as xtp,
            tc.tile_pool(name="tpool", bufs=1) as tpool,
            tc.tile_pool(name="ttpool", bufs=2) as ttpool,
            tc.tile_pool(name="opool", bufs=o["o_bufs"]) as opool,
            tc.tile_pool(name="stats", bufs=2) as stats,
            tc.tile_pool(name="stgp", bufs=2) as stgp,
            tc.tile_pool(name="pe", bufs=1, space="PSUM") as psum_e,
            tc.tile_pool(name="pxt", bufs=o["psx_bufs"], space="PSUM") as psum_xt,
            tc.tile_pool(name="pacc", bufs=o["acc_bufs"], space="PSUM") as psum_acc,
        ):
            identf = consts.tile([P, P], F32)
            make_identity(nc, identf)
            ident8 = consts.tile([P, P], FP8)
            nc.scalar.copy(ident8[:, :], identf[:, :])
            ident16 = consts.tile([P, P], BF16)
            nc.scalar.copy(ident16[:, :], identf[:, :])
            g_sb = consts.tile([1, 1], F32)
            nc.sync.dma_start(g_sb[:, :], g_in[:].rearrange("(a b) -> a b", a=1))
            g_col = consts.tile([P, 1], F32)
            nc.gpsimd.partition_broadcast(g_col[:, :], g_sb[:1, :1])

            def emit_load(b):
                """DMA x_b in and cast to fp8.  Returns (X, X8) tiles."""
                x_b = x_in[b].rearrange("(co p) n -> p co n", p=P)
                X = xpool.tile([P, CO, N], F32, tag="X", name="X")
                # first 128 columns land alone so transposes start early
                nc.sync.dma_start(X[:, :, 0:P], x_b[:, :, 0:P])
                nc.sync.dma_start(X[:, :, P:512], x_b[:, :, P:512])
                for nf in range(1, NF):
                    s = slice(nf * 512, (nf + 1) * 512)
                    nc.sync.dma_start(X[:, :, s], x_b[:, :, s])
                X8 = x8pool.tile([P, CO, N], FP8, tag="X8", name="X8")
                for nf in range(NF):
                    s = slice(nf * 512, (nf + 1) * 512)
                    eng = o["cast_engines"][nf % len(o["cast_engines"])]
                    if eng == "gpsimd":
                        nc.gpsimd.tensor_copy(X8[:, :, s], X[:, :, s])
                    elif eng == "vector":
                        nc.vector.tensor_copy(X8[:, :, s], X[:, :, s])
                    else:
                        nc.scalar.copy(X8[:, :, s], X[:, :, s])
                return X, X8

            loop_ctx = (
                tc.For_i(0, reps, 1) if reps > 1 else contextlib.nullcontext()
            )
            with loop_ctx:
                pend_load = emit_load(0)
                for b in range(Bs):
                    X, X8 = pend_load
                    y_b = y_out[b].rearrange("(co p) n -> p co n", p=P)

                    # ---- energy = x @ x.T (contraction over n on partitions)
                    # E is symmetric: compute upper-triangular blocks only
                    # (row ic covers columns >= ic*P), mirror the rest after.
                    E = psum_e.tile([P, CO, C], F32, tag="E")

                    def emit_trans(kp, X8=X8):
                        xt_k = xtp.tile([P, 2, C], FP8, tag="xt", name="xt_k")
                        for ko in range(2):
                            kc = 2 * kp + ko
                            ks = slice(kc * P, (kc + 1) * P)
                            ps_x = psum_xt.tile([P, C], F32, tag="psx",
                                                name="ps_x")
                            for co in range(CO):
                                nc.tensor.matmul(
                                    ps_x[:, co * P:(co + 1) * P],
                                    X8[:, co, ks], ident8[:, :],
                                    start=True, stop=True,
                                )
                            nc.scalar.copy(xt_k[:, ko, :], ps_x[:, :])
                        return xt_k

                    def emit_mm1(kp, xt_k, E=E):
                        for ic in range(CO):
                            if o["dr1"]:
                                nc.tensor.matmul(
                                    E[:, ic, ic * P:],
                                    xt_k[:, :, ic * P:(ic + 1) * P],
                                    xt_k[:, :, ic * P:],
                                    start=(kp == 0),
                                    stop=(kp == KP - 1),
                                    perf_mode=DR,
                                )
                            else:
                                for ko in range(2):
                                    nc.tensor.matmul(
                                        E[:, ic, ic * P:],
                                        xt_k[:, ko, ic * P:(ic + 1) * P],
                                        xt_k[:, ko, ic * P:],
                                        start=(kp == 0 and ko == 0),
                                        stop=(kp == KP - 1 and ko == 1),
                                    )

                    depth = o["pipe1"]
                    pend = {}
                    for kp in range(KP):
                        pend[kp] = emit_trans(kp)
                        if kp >= depth:
                            emit_mm1(kp - depth, pend.pop(kp - depth))
                    for kp in range(KP - depth, KP):
                        emit_mm1(kp, pend.pop(kp))

                    # mirror E[jc, ic] = E[ic, jc].T for ic < jc
                    # (stage must be bf16: |E| can exceed fp8e4's +-240)
                    for jc in range(1, CO):
                        for ic in range(jc):
                            stg = stgp.tile([P, P], BF16, tag="mirror_stage")
                            nc.scalar.copy(
                                stg[:, :], E[:, ic, jc * P:(jc + 1) * P]
                            )
                            nc.tensor.matmul(
                                E[:, jc, ic * P:(ic + 1) * P],
                                stg[:, :], ident16[:, :],
                                start=True, stop=True,
                                skip_group_check=True,
                            )

                    # ---- softmax: t = exp(mn - E), Z row-sum fused ----
                    mn = stats.tile([P, CO], F32, tag="mn")
                    zs = stats.tile([P, CO], F32, tag="zs")
                    rg = stats.tile([P, CO], F32, tag="rg")
                    tS = tpool.tile([P, CO, C], FP8, tag="t")
                    for ic in range(CO):
                        nc.vector.tensor_reduce(
                            mn[:, ic:ic + 1], E[:, ic, :], AX.X, ALU.min
                        )
                    for ic in range(CO):
                        nc.scalar.activation(
                            tS[:, ic, :], E[:, ic, :], AF.Exp,
                            bias=mn[:, ic:ic + 1], scale=-1.0,
                            accum_out=zs[:, ic:ic + 1],
                        )
                    nc.vector.reciprocal(rg[:, :], zs[:, :])

                    # ---- tT8[j, i] = gamma * t[i, j] ----
                    tT8 = ttpool.tile([P, CO, C], FP8, tag="tT")
                    for jc in range(CO):
                        ps_t = psum_acc.tile([P, C], F32, tag="acc")
                        for ic in range(CO):
                            nc.tensor.matmul(
                                ps_t[:, ic * P:(ic + 1) * P],
                                tS[:, ic, jc * P:(jc + 1) * P],
                                ident8[:, :],
                                start=True, stop=True,
                            )
                        nc.scalar.activation(
                            tT8[:, jc, :], ps_t[:, :], AF.Copy,
                            bias=0.0, scale=g_col[:, :1],
                        )

                    # next batch's input DMA + cast go ahead of this batch's
                    # output DMAs in the Sync/engine FIFOs
                    if b + 1 < Bs:
                        pend_load = emit_load(b + 1)

                    # ---- out = (gamma*att) @ x, * 1/Z + residual ----
                    for ic in range(CO):
                        for nfp in range(NF // 2):
                            ot = opool.tile([P, 1024], F32, tag="o")
                            for h in range(2):
                                nf = 2 * nfp + h
                                ns = slice(nf * 512, (nf + 1) * 512)
                                ps2 = psum_acc.tile([P, 512], F32, tag="acc")
                                if o["dr2"]:
                                    for q in range(2):
                                        nc.tensor.matmul(
                                            ps2[:, :],
                                            tT8[:, 2 * q:2 * q + 2,
                                                ic * P:(ic + 1) * P],
                                            X8[:, 2 * q:2 * q + 2, ns],
                                            start=(q == 0), stop=(q == 1),
                                            perf_mode=DR,
                                        )
                                else:
                                    for jc in range(CO):
                                        nc.tensor.matmul(
                                            ps2[:, :],
                                            tT8[:, jc, ic * P:(ic + 1) * P],
                                            X8[:, jc, ns],
                                            start=(jc == 0),
                                            stop=(jc == CO - 1),
                                        )
                                nc.vector.scalar_tensor_tensor(
                                    ot[:, h * 512:(h + 1) * 512], ps2[:, :],
                                    rg[:, ic:ic + 1], X[:, ic, ns],
                                    op0=ALU.mult, op1=ALU.add,
                                )
                            nc.sync.dma_start(
                                y_b[:, ic, nfp * 1024:(nfp + 1) * 1024],
                                ot[:, :],
                            )

    nc.compile()
    return nc


def get_nc(Bs=4, C=512, N=4096, reps=1, **opts):
    key = (Bs, C, N, reps, tuple(sorted(opts.items())))
    if key not in _CACHE:
        _CACHE[key] = _build(Bs, C, N, reps, **opts)
    return _CACHE[key]


def kernel(x, gamma):
    """Full inputs in, full output out. x [32, 512, 4096] f32, gamma [1] f32."""
    from concourse.bass_utils import run_bass_kernel_spmd

    x = np.ascontiguousarray(np.asarray(x, dtype=np.float32))
    gamma = np.ascontiguousarray(np.asarray(gamma, dtype=np.float32))
    B, C, N = x.shape
    n_cores = 8
    assert B % n_cores == 0
    Bs = B // n_cores

    nc = get_nc(Bs, C, N)
    in_maps = [
        {"x": x[i * Bs:(i + 1) * Bs], "gamma": gamma} for i in range(n_cores)
    ]
    res = run_bass_kernel_spmd(nc, in_maps, core_ids=list(range(n_cores)))
    return np.concatenate([r["y"] for r in res.results], axis=0)


# revision 8
# speedup vs baseline: 1.0295x; 1.0295x over previous
"""CAM (channel attention) module kernel for Trainium2 (Bass/Tile) — v4.

Reference computation (per batch b):
    energy  = x_b @ x_b.T                      # [C, C], contraction over N
    att     = softmax(rowmax(energy) - energy) # row-wise over last axis
    out     = att @ x_b                        # [C, N]
    y_b     = gamma * out + x_b

Sharding: data-parallel over B across 8 NeuronCores (B=32 -> 4 per core),
gamma replicated, full CxC attention per core.

Identity used: softmax(rowmax(E) - E)[i,j] = exp(mn[i] - E[i,j]) / Z[i]
with mn[i] = min_j E[i,j], Z[i] = sum_j exp(mn[i] - E[i,j]).

Evidence-driven structure (NTFF profiles of v1..v3):
  * all PE transposes are REGULAR matmuls against an fp8 identity
    (moving operand = identity): ~56-81ns issue vs ~330-480ns
    transpose-mode, and they keep the PE clock-gate (HAM) warm.
  * fp8e4 operands (TRN e4m3, +-240 range; |x|<~6); mm1+mm2 DoubleRow.
  * batch software pipeline: batch b+1's input DMA + fp8 cast are emitted
    BEFORE batch b's mm2/output block (the Sync FIFO's out-DMAs otherwise
    head-of-line-block next input: ~15us PE hole per batch in v2).
  * x->fp8 cast chunks: 0-2 on GpSimd (starts as DMA lands, otherwise
    idle), 3-7 on ScalarE (runs in its free mm2-phase window).  DVE gets
    none: its STT queue must not be head-of-line blocked (v3 lesson).
  * xt evacuations: every 3rd on DVE -- ScalarE alone was rate-matched
    with PE consumption and stalled the transposes (v3: ~8us).
  * transpose PSUM staging rotates over psx+acc pools (4 banks) to
    absorb ScalarE burst latency.
  * mm2's last ic-group is DEFERRED into the next batch, emitted right
    after mm1: it fills the softmax-latency bubble (mirror->min->exp
    chain, ~3.5us) with useful PE work.
  * gamma folded into the t^T evacuation (ACT scale); output evac is one
    scalar_tensor_tensor (mul 1/Z, add residual); outputs DMA'd in
    1024-col pairs.

Layouts per batch (P=128 partitions):
    X   [P, CO, N]  f32  c-natural  (c = co*P + p)        -- DMA from DRAM
    X8  [P, CO, N]  fp8  cast of X  (mm2 moving operand)
    xt8 [P, 2, C]   fp8  per k-pair, n on partitions      -- PE transpose
    E   [P, CO, C]  PSUM f32, i on partitions             -- mm1 (DoubleRow)
    tS  [P, CO, C]  fp8  exp(mn - E); Z row-sum fused     -- ScalarE
    tT8 [P, CO, C]  fp8  gamma * t^T, j on partitions     -- PE + ACT scale
    out [P, 1024]   f32  (tT8.T @ X8) * (1/Z[i]) + X      -- mm2 (DoubleRow)
"""

import contextlib

import numpy as np

P = 128

_CACHE = {}


DEFAULT_OPTS = dict(
    dr1=True,        # DoubleRow for mm1 (energy)
    dr2=True,        # DoubleRow for mm2 (att @ x)
    xt_bufs=6,       # xt8 k-pair SBUF tiles
    psx_bufs=2,      # PSUM banks in the dedicated transpose-staging pool
    acc_bufs=2,      # PSUM banks shared by staging/t-transpose/matmul-2
    o_bufs=4,        # output staging tiles [P, 1024]
    pipe1=2,         # mm1 trails transposes by this many k-pairs
    defer_ic=1,      # mm2 ic-groups deferred into the next batch
    evac_dve_mod=3,  # every Nth xt evac goes to DVE (0 = none)
    cast_engines=("gpsimd", "gpsimd", "gpsimd", "scalar",
                  "scalar", "scalar", "scalar", "scalar"),
)


def _build(Bs, C, N, reps=1, **opts):
    import concourse.bass as bass  # noqa: F401
    import concourse.tile as tile
    import concourse.mybir as mybir
    from concourse import bacc
    from concourse.masks import make_identity

    o = dict(DEFAULT_OPTS)
    o.update(opts)

    F32 = mybir.dt.float32
    BF16 = mybir.dt.bfloat16
    FP8 = mybir.dt.float8e4
    AF = mybir.ActivationFunctionType
    ALU = mybir.AluOpType
    AX = mybir.AxisListType
    DR = mybir.MatmulPerfMode.DoubleRow

    assert C == 4 * P and N % 1024 == 0
    CO = C // P          # i/j chunks of 128
    KC = N // P          # n chunks of 128 (contraction for energy)
    KP = KC // 2         # n chunk-pairs (DoubleRow contraction of 256)
    NF = N // 512        # n chunks of 512

    nc = bacc.Bacc(None, target_bir_lowering=False, debug=False)
    x_in = nc.dram_tensor("x", [Bs, C, N], F32, kind="ExternalInput")
    g_in = nc.dram_tensor("gamma", [1], F32, kind="ExternalInput")
    y_out = nc.dram_tensor("y", [Bs, C, N], F32, kind="ExternalOutput")

    with tile.TileContext(nc) as tc:
        with (
            tc.tile_pool(name="consts", bufs=1) as consts,
            tc.tile_pool(name="xpool", bufs=2) as xpool,
            tc.tile_pool(name="x8pool", bufs=2) as x8pool,
            tc.tile_pool(name="xtp", bufs=o["xt_bufs"]) as xtp,
            tc.tile_pool(name="tpool", bufs=1) as tpool,
            tc.tile_pool(name="ttpool", bufs=2) as ttpool,
            tc.tile_pool(name="opool", bufs=o["o_bufs"]) as opool,
            tc.tile_pool(name="stats", bufs=2) as stats,
            tc.tile_pool(name="stgp", bufs=2) as stgp,
            tc.tile_pool(name="pe", bufs=1, space="PSUM") as psum_e,
            tc.tile_pool(name="pxt", bufs=o["psx_bufs"], space="PSUM") as psum_xt,
            tc.tile_pool(name="pacc", bufs=o["acc_bufs"], space="PSUM") as psum_acc,
        ):
            identf = consts.tile([P, P], F32)
            make_identity(nc, identf)
            ident8 = consts.tile([P, P], FP8)
            nc.scalar.copy(ident8[:, :], identf[:, :])
            ident16 = consts.tile([P, P], BF16)
            nc.scalar.copy(ident16[:, :], identf[:, :])
            g_sb = consts.tile([1, 1], F32)
            nc.sync.dma_start(g_sb[:, :], g_in[:].rearrange("(a b) -> a b", a=1))
            g_col = consts.tile([P, 1], F32)
            nc.gpsimd.partition_broadcast(g_col[:, :], g_sb[:1, :1])

            def emit_load(b):
                """DMA x_b in and cast to fp8.  Returns (X, X8) tiles."""
                x_b = x_in[b].rearrange("(co p) n -> p co n", p=P)
                X = xpool.tile([P, CO, N], F32, tag="X", name="X")
                for nf in range(NF):
                    s = slice(nf * 512, (nf + 1) * 512)
                    nc.sync.dma_start(X[:, :, s], x_b[:, :, s])
                X8 = x8pool.tile([P, CO, N], FP8, tag="X8", name="X8")
                for nf in range(NF):
                    s = slice(nf * 512, (nf + 1) * 512)
                    eng = o["cast_engines"][nf % len(o["cast_engines"])]
                    if eng == "gpsimd":
                        nc.gpsimd.tensor_copy(X8[:, :, s], X[:, :, s])
                    elif eng == "vector":
                        nc.vector.tensor_copy(X8[:, :, s], X[:, :, s])
                    else:
                        nc.scalar.copy(X8[:, :, s], X[:, :, s])
                return X, X8

            def emit_mm2(ctx, ic_list):
                """mm2 + output evac/DMA for the given ic row-chunks."""
                X, X8, tT8, rg, y_b = ctx
                for ic in ic_list:
                    for nfp in range(NF // 2):
                        ot = opool.tile([P, 1024], F32, tag="o")
                        for h in range(2):
                            nf = 2 * nfp + h
                            ns = slice(nf * 512, (nf + 1) * 512)
                            ps2 = psum_acc.tile([P, 512], F32, tag="acc")
                            if o["dr2"]:
                                for q in range(2):
                                    nc.tensor.matmul(
                                        ps2[:, :],
                                        tT8[:, 2 * q:2 * q + 2,
                                            ic * P:(ic + 1) * P],
                                        X8[:, 2 * q:2 * q + 2, ns],
                                        start=(q == 0), stop=(q == 1),
                                        perf_mode=DR,
                                    )
                            else:
                                for jc in range(CO):
                                    nc.tensor.matmul(
                                        ps2[:, :],
                                        tT8[:, jc, ic * P:(ic + 1) * P],
                                        X8[:, jc, ns],
                                        start=(jc == 0),
                                        stop=(jc == CO - 1),
                                    )
                            nc.vector.scalar_tensor_tensor(
                                ot[:, h * 512:(h + 1) * 512], ps2[:, :],
                                rg[:, ic:ic + 1], X[:, ic, ns],
                                op0=ALU.mult, op1=ALU.add,
                            )
                        nc.sync.dma_start(
                            y_b[:, ic, nfp * 1024:(nfp + 1) * 1024],
                            ot[:, :],
                        )

            loop_ctx = (
                tc.For_i(0, reps, 1) if reps > 1 else contextlib.nullcontext()
            )
            with loop_ctx:
                pend_load = emit_load(0)
                pend_mm2 = None   # (ctx, ic_list) deferred from prev batch
                n_defer = min(o["defer_ic"], CO - 1)
                for b in range(Bs):
                    X, X8 = pend_load
                    y_b = y_out[b].rearrange("(co p) n -> p co n", p=P)

                    # ---- energy = x @ x.T (contraction over n on partitions)
                    # E is symmetric: compute upper-triangular blocks only
                    # (row ic covers columns >= ic*P), mirror the rest after.
                    E = psum_e.tile([P, CO, C], F32, tag="E")

                    def emit_trans(kp, X8=X8):
                        xt_k = xtp.tile([P, 2, C], FP8, tag="xt", name="xt_k")
                        for ko in range(2):
                            kc = 2 * kp + ko
                            ks = slice(kc * P, (kc + 1) * P)
                            i = 2 * kp + ko
                            pool = psum_xt if i % 2 == 0 else psum_acc
                            tag = "psx" if i % 2 == 0 else "acc"
                            ps_x = pool.tile([P, C], F32, tag=tag, name="ps_x")
                            for co in range(CO):
                                nc.tensor.matmul(
                                    ps_x[:, co * P:(co + 1) * P],
                                    X8[:, co, ks], ident8[:, :],
                                    start=True, stop=True,
                                )
                            m = o["evac_dve_mod"]
                            if m and i % m == m - 1:
                                nc.vector.tensor_copy(xt_k[:, ko, :],
                                                      ps_x[:, :])
                            else:
                                nc.scalar.copy(xt_k[:, ko, :], ps_x[:, :])
                        return xt_k

                    def emit_mm1(kp, xt_k, E=E):
                        for ic in range(CO):
                            if o["dr1"]:
                                nc.tensor.matmul(
                                    E[:, ic, ic * P:],
                                    xt_k[:, :, ic * P:(ic + 1) * P],
                                    xt_k[:, :, ic * P:],
                                    start=(kp == 0),
                                    stop=(kp == KP - 1),
                                    perf_mode=DR,
                                )
                            else:
                                for ko in range(2):
                                    nc.tensor.matmul(
                                        E[:, ic, ic * P:],
                                        xt_k[:, ko, ic * P:(ic + 1) * P],
                                        xt_k[:, ko, ic * P:],
                                        start=(kp == 0 and ko == 0),
                                        stop=(kp == KP - 1 and ko == 1),
                                    )

                    depth = o["pipe1"]
                    pend = {}
                    for kp in range(KP):
                        pend[kp] = emit_trans(kp)
                        if kp >= depth:
                            emit_mm1(kp - depth, pend.pop(kp - depth))
                    for kp in range(KP - depth, KP):
                        emit_mm1(kp, pend.pop(kp))

                    # deferred mm2 tail of the previous batch: fills the
                    # PE bubble while this batch's softmax chain runs
                    if pend_mm2 is not None:
                        emit_mm2(*pend_mm2)
                        pend_mm2 = None

                    # mirror E[jc, ic] = E[ic, jc].T for ic < jc
                    # (stage must be bf16: |E| can exceed fp8e4's +-240)
                    for jc in range(1, CO):
                        for ic in range(jc):
                            stg = stgp.tile([P, P], BF16, tag="mirror_stage")
                            nc.scalar.copy(
                                stg[:, :], E[:, ic, jc * P:(jc + 1) * P]
                            )
                            nc.tensor.matmul(
                                E[:, jc, ic * P:(ic + 1) * P],
                                stg[:, :], ident16[:, :],
                                start=True, stop=True,
                                skip_group_check=True,
                            )

                    # ---- softmax: t = exp(mn - E), Z row-sum fused ----
                    mn = stats.tile([P, CO], F32, tag="mn")
                    zs = stats.tile([P, CO], F32, tag="zs")
                    rg = stats.tile([P, CO], F32, tag="rg")
                    tS = tpool.tile([P, CO, C], FP8, tag="t")
                    for ic in range(CO):
                        nc.vector.tensor_reduce(
                            mn[:, ic:ic + 1], E[:, ic, :], AX.X, ALU.min
                        )
                    for ic in range(CO):
                        nc.scalar.activation(
                            tS[:, ic, :], E[:, ic, :], AF.Exp,
                            bias=mn[:, ic:ic + 1], scale=-1.0,
                            accum_out=zs[:, ic:ic + 1],
                        )
                    nc.vector.reciprocal(rg[:, :], zs[:, :])

                    # ---- tT8[j, i] = gamma * t[i, j] ----
                    tT8 = ttpool.tile([P, CO, C], FP8, tag="tT")
                    for jc in range(CO):
                        ps_t = psum_acc.tile([P, C], F32, tag="acc")
                        for ic in range(CO):
                            nc.tensor.matmul(
                                ps_t[:, ic * P:(ic + 1) * P],
                                tS[:, ic, jc * P:(jc + 1) * P],
                                ident8[:, :],
                                start=True, stop=True,
                            )
                        nc.scalar.activation(
                            tT8[:, jc, :], ps_t[:, :], AF.Copy,
                            bias=0.0, scale=g_col[:, :1],
                        )

                    # next batch's input DMA + cast go ahead of this batch's
                    # output DMAs in the Sync/engine FIFOs
                    if b + 1 < Bs:
                        pend_load = emit_load(b + 1)

                    # ---- out = (gamma*att) @ x, * 1/Z + residual ----
                    ctx = (X, X8, tT8, rg, y_b)
                    if b + 1 < Bs and n_defer > 0:
                        emit_mm2(ctx, list(range(CO - n_defer)))
                        pend_mm2 = (ctx, list(range(CO - n_defer, CO)))
                    else:
                        emit_mm2(ctx, list(range(CO)))

    nc.compile()
    return nc


def get_nc(Bs=4, C=512, N=4096, reps=1, **opts):
    key = (Bs, C, N, reps, tuple(sorted(opts.items())))
    if key not in _CACHE:
        _CACHE[key] = _build(Bs, C, N, reps, **opts)
    return _CACHE[key]


def kernel(x, gamma):
    """Full inputs in, full output out. x [32, 512, 4096] f32, gamma [1] f32."""
    from concourse.bass_utils import run_bass_kernel_spmd

    x = np.ascontiguousarray(np.asarray(x, dtype=np.float32))
    gamma = np.ascontiguousarray(np.asarray(gamma, dtype=np.float32))
    B, C, N = x.shape
    n_cores = 8
    assert B % n_cores == 0
    Bs = B // n_cores

    nc = get_nc(Bs, C, N)
    in_maps = [
        {"x": x[i * Bs:(i + 1) * Bs], "gamma": gamma} for i in range(n_cores)
    ]
    res = run_bass_kernel_spmd(nc, in_maps, core_ids=list(range(n_cores)))
    return np.concatenate([r["y"] for r in res.results], axis=0)
